# revision 11
# baseline (speedup 1.0000x reference)
"""Trainium2 Bass kernel for the BiDAF-style trilinear attention module.

Math (per batch b; bf16 operands, f32 PSUM accumulate — harness gate is
rel_err < 2e-2):
  w_c, w_q, w_cq = attn_w[0:256], attn_w[256:512], attn_w[512:768]
  sim[l,q] = ctx[l]·w_c + qry[q]·w_q + (ctx[l]*w_cq)·qry[q] + attn_b
  alpha    = softmax_q(sim)                      (masks are all-ones)
  a        = alpha @ qry                         [L, D]
  q2c      = max_q(sim);  beta = softmax_l(q2c)
  bvec     = beta @ ctx                          [D]
  out      = concat([ctx, a, ctx*a, ctx*bvec])   [L, 4D]

The device computes ONLY the parts that need the big L×Q similarity
matrix; everything cheap rides the host gather step, so HBM traffic
per core drops to 9.1 MB and per-engine work per batch stays under the
DMA budget:
  * attn_b cancels in both softmaxes -> dropped entirely.
  * s_c[l] = ctx[l]·w_c is constant along the softmax_q axis -> alpha
    does not need it; the host adds it back for the beta softmax.
  * qext = qryT * w_cq (the sim stationary operand) and s_q = qry·w_q
    are precomputed on the HOST: qext ships in place of qryT, s_q
    ships as a tiny f32 tensor that becomes the ACT exp bias.
  * sim' is computed TRANSPOSED (simT[q,l]) with a 512-wide moving dim:
    2 accumulating matmuls per 512-block; alphaU = exp(simT' + s_q)
    straight from PSUM (ACT, per-partition bias).
  * alpha rowsum rides as a 257th ones-column of the a-matmul rhs; the
    device ships UNNORMALIZED a' and the rowsum s; the host divides.
  * the a-matmuls write PAIRS of l-tiles into one 2-bank PSUM tile
    (bank-aligned 257-col slices) so each PSUM->SBUF copy covers two
    tiles — half the per-op fixed overhead on ACT/DVE.
  * eb[l] = max_q alphaU = exp(q2c[l] - s_c[l] - s_q-less terms) -> one
    PE transpose per 128-l tile (4 per PSUM bank) + one free-axis DVE
    max per block; host: beta = softmax_l(log(eb) + s_c), bvec = beta@ctx.
  * out[:, :, 0:D] == ctx (exact), ctx*a and ctx*bvec are elementwise
    f32 products done on the host during the gather.

I/O layout: the host packs per-batch inputs (ctx transposed, qext,
qry+ones) into ONE partition-major bf16 buffer so each batch is a
single fully-contiguous 0.63 MB DMA (5.1 KB/partition lines); the
device writes one 2064-col bf16 output buffer per batch ([a'|s] tiles
+ eb), shipped as two ~260 KB DMAs (2 KB lines) — per-pair DMAs on the
last batch to shorten the drain tail.

Sharding: data-parallel over batch, 8 batches per NeuronCore x 8 cores.
"""

import sys

sys.path.insert(0, "/opt/trn_rl_repo")

from contextlib import ExitStack

import numpy as np
import ml_dtypes

import concourse.bass as bass
import concourse.bacc as bacc
import concourse.tile as tile
from concourse import mybir
from concourse.masks import make_identity
from concourse.bass_utils import run_bass_kernel_spmd

B, L, Q, D = 64, 1024, 128, 256
NCORES = 8
BPC = B // NCORES          # batches per core
NT = L // 128              # 128-row l-tiles per batch
BW = 512                   # sim block width (l columns per PSUM bank)
NBLK = L // BW             # sim blocks per batch
TPB = BW // 128            # l-tiles per sim block
F32 = mybir.dt.float32
BF16 = mybir.dt.bfloat16
EXP = mybir.ActivationFunctionType.Exp
X = mybir.AxisListType.X
NPBF16 = ml_dtypes.bfloat16

# packed input layout (per batch, per partition, bf16 elements):
#   [ ctxT (c,l) | qext = (qry*w_cq)T (c,q) | qry row + ones ]
O_CT = 0                   # ctxT [128, 2, L]
O_QE = O_CT + 2 * L        # qextT [128, 2, Q]
O_QN = O_QE + 2 * Q        # [qry | 1] [128, D+1]
NIN = O_QN + D + 1         # 2561

# output layout (per batch, per partition, bf16 elements):
#   [ NT tiles of [a'(256) | rowsum(1)] | eb (NT) ]
TW = D + 1                 # a-tile width incl rowsum rider
O_EB = NT * TW             # 2056
NOUT = O_EB + NT           # 2064
BLKW = TPB * TW            # output cols per sim block (1028)


def build_module() -> bass.Bass:
    # Bacc (not plain Bass): its compile() pass splits multi-sem waits into
    # event semaphores — walrus's LDWEIGHTS struct only carries one wait.
    nc = bacc.Bacc("TRN2", target_bir_lowering=False)
    in_t = nc.declare_dram_parameter("inpack", [BPC, 128, NIN], BF16, isOutput=False)
    sq_t = nc.declare_dram_parameter("sq_all", [128, BPC], F32, isOutput=False)
    out_t = nc.declare_dram_parameter("out3", [BPC, 128, NOUT], BF16,
                                      isOutput=True)

    with tile.TileContext(nc) as tc, ExitStack() as ctx:
        consts = ctx.enter_context(tc.tile_pool(name="consts", bufs=1))
        sb = ctx.enter_context(tc.tile_pool(name="sb", bufs=6))
        # dedicated SBUF buffers for every batch — zero pool-reuse waits
        big = ctx.enter_context(tc.tile_pool(name="big", bufs=BPC))
        ob = ctx.enter_context(tc.tile_pool(name="ob", bufs=BPC))
        # PSUM: 8 banks exactly — sim(2x1) + at(2x1) + a-pairs(2x2)
        ps_sim = ctx.enter_context(tc.tile_pool(name="ps_sim", bufs=2, space="PSUM"))
        ps_at = ctx.enter_context(tc.tile_pool(name="ps_at", bufs=2, space="PSUM"))
        ps_a = ctx.enter_context(tc.tile_pool(name="ps_a", bufs=2, space="PSUM"))

        identity = consts.tile([128, 128], BF16)
        make_identity(nc, identity)
        # s_q for all batches as ACT-bias columns, one tiny DMA
        sqsb = consts.tile([128, BPC], F32)
        nc.sync.dma_start(out=sqsb, in_=sq_t[:, :])

        # PE warm-up: dummy matmuls while the first input DMAs are in
        # flight, so the HAM clock ramp completes before the real work.
        wtile = ps_at.tile([128, 128], F32, tag="at", name="warmup")
        for _ in range(24):
            nc.tensor.matmul(wtile, lhsT=identity, rhs=identity,
                             start=True, stop=True)

        def dma_in(b):
            # traffic is spread over two independent DMA rings: batches
            # 0-1 ride the Sync HWDGE ring (idle until the first outputs),
            # the rest ride the GpSimd software-DGE ring, so input
            # prefetch never serializes behind output transfers.
            eng = nc.sync if b < 2 else nc.gpsimd
            ibuf = big.tile([128, NIN], BF16, tag="ibuf", name=f"ibuf{b}")
            if b == 0:
                # split so the first sim block starts before qn lands
                eng.dma_start(out=ibuf[:, O_CT:O_QN], in_=in_t[b][:, O_CT:O_QN])
                eng.dma_start(out=ibuf[:, O_QN:NIN], in_=in_t[b][:, O_QN:NIN])
            else:
                eng.dma_start(out=ibuf, in_=in_t[b])
            return {
                "ct2": ibuf[:, O_CT:O_QE].rearrange("p (c l) -> p c l", c=2),
                "qe2": ibuf[:, O_QE:O_QN].rearrange("p (c q) -> p c q", c=2),
                "qn": ibuf[:, O_QN:NIN],
            }

        def sim_block(b, st, j):
            ct2, qe2 = st["ct2"], st["qe2"]
            lo, hi = j * BW, (j + 1) * BW
            # simT'[q, l] = (qry*w_cq)·ctx — no s_c fold (host adds it)
            sim_ps = ps_sim.tile([128, BW], F32, tag="sim", name=f"sim{b}_{j}")
            nc.tensor.matmul(sim_ps, lhsT=qe2[:, 0, :], rhs=ct2[:, 0, lo:hi],
                             start=True, stop=False)
            nc.tensor.matmul(sim_ps, lhsT=qe2[:, 1, :], rhs=ct2[:, 1, lo:hi],
                             start=False, stop=True)
            # alphaU[q, l] = exp(simT' + s_q) — unnormalized alpha^T
            alphaU = sb.tile([128, BW], BF16, tag="alpha", name=f"alpha{b}_{j}")
            nc.scalar.activation(out=alphaU, in_=sim_ps, func=EXP,
                                 bias=sqsb[:, b : b + 1])
            st[f"alpha{j}"] = alphaU

        def tail_block(b, st, j):
            qn, obuf, alphaU = st["qn"], st["obuf"], st[f"alpha{j}"]
            at_ps = st["at_ps"]
            t0 = j * TPB
            # eb[l] = max over q of alphaU — PE transposes of the whole
            # batch pack one PSUM bank, then a SINGLE per-batch DVE max
            for i in range(TPB):
                nc.tensor.transpose(at_ps[:, t0 + i, :],
                                    alphaU[:, i * 128 : (i + 1) * 128], identity)
            if j == NBLK - 1:
                nc.vector.reduce_max(obuf[:, O_EB : O_EB + NT], at_ps, axis=X)
            for p in range(TPB // 2):
                # a-matmul PAIR: two l-tiles into one 2-bank PSUM tile,
                # each [a' | rowsum] slice bank-aligned (257 of 512 cols)
                a_ps = ps_a.tile([128, 2, 512], F32, tag="a",
                                 name=f"a_ps{b}_{j}_{p}")
                for i in range(2):
                    asl = alphaU[:, (2 * p + i) * 128 : (2 * p + i + 1) * 128]
                    nc.tensor.matmul(a_ps[:, i, 0:TW], lhsT=asl, rhs=qn,
                                     start=True, stop=True)
                # one 514-col PSUM->SBUF copy per pair, ACT/DVE alternating
                dst = obuf[:, (t0 + 2 * p) * TW : (t0 + 2 * p + 2) * TW]
                dst = dst.rearrange("p (i w) -> p i w", i=2)
                if p == 0:
                    nc.scalar.copy(dst, a_ps[:, :, 0:TW])
                else:
                    nc.vector.tensor_copy(dst, a_ps[:, :, 0:TW])
                if b == BPC - 1:
                    # drain tail: ship each pair as soon as it is copied
                    lo = (t0 + 2 * p) * TW
                    hi = (t0 + 2 * p + 2) * TW if (j, p) != (NBLK - 1, 1) \
                        else NOUT
                    nc.sync.dma_start(out=out_t[b][:, lo:hi],
                                      in_=obuf[:, lo:hi])
            if b < BPC - 1:
                # per-block output DMA (block 1 carries the eb tail columns)
                lo = j * BLKW
                hi = (j + 1) * BLKW if j < NBLK - 1 else NOUT
                nc.sync.dma_start(out=out_t[b][:, lo:hi], in_=obuf[:, lo:hi])

        # Software pipeline: input DMAs prefetched two batches ahead; both
        # sim blocks are emitted before either tail so the PE never waits
        # on the ACT exp.
        # all inputs issued up front (dedicated buffers -> no waits)
        states = {b: dma_in(b) for b in range(BPC)}
        for b in range(BPC):
            st = states.pop(b)
            st["obuf"] = ob.tile([128, NOUT], BF16, tag="obuf",
                                 name=f"obuf{b}")
            st["at_ps"] = ps_at.tile([128, NT, 128], BF16, tag="at",
                                     name=f"at{b}")
            sim_block(b, st, 0)
            sim_block(b, st, 1)
            tail_block(b, st, 0)
            tail_block(b, st, 1)

    nc.finalize()
    return nc


def make_in_maps(context: np.ndarray, query: np.ndarray, attn_w: np.ndarray):
    """Shard + lay out the full f32 inputs for the 8 cores: one packed
    partition-major bf16 buffer per batch (see layout comment up top),
    plus the per-batch s_q bias columns in f32."""
    w_cq = attn_w[2 * D :].astype(np.float32)
    w_q = attn_w[D : 2 * D].astype(np.float32)
    ctx_b = context.astype(NPBF16)
    qe_b = (query * w_cq).astype(NPBF16)       # qext, host-side
    qry_b = query.astype(NPBF16)
    sq = (query.astype(np.float32) @ w_q).astype(np.float32)   # [B, Q]
    maps = []
    for i in range(NCORES):
        sl = slice(i * BPC, (i + 1) * BPC)
        c = ctx_b[sl]                                          # [BPC, L, D]
        qe = qe_b[sl]                                          # [BPC, Q, D]
        c2 = np.ascontiguousarray(c.transpose(0, 2, 1)).reshape(
            BPC, 2, 128, L).transpose(0, 2, 1, 3).reshape(BPC, 128, 2 * L)
        qT = np.ascontiguousarray(qe.transpose(0, 2, 1)).reshape(
            BPC, 2, 128, Q).transpose(0, 2, 1, 3).reshape(BPC, 128, 2 * Q)
        qn = np.concatenate([qry_b[sl], np.ones((BPC, Q, 1), NPBF16)], axis=2)
        inpack = np.ascontiguousarray(
            np.concatenate([c2, qT, qn], axis=2))              # [BPC, 128, NIN]
        maps.append({"inpack": inpack,
                     "sq_all": np.ascontiguousarray(sq[sl].T)})  # [128, BPC]
    return maps


def assemble(context: np.ndarray, attn_w: np.ndarray, results) -> np.ndarray:
    """Gather per-core [a'|s] tiles + eb, normalize a, rebuild the beta
    path (softmax_l(log eb + s_c), bvec = beta@ctx) and the elementwise
    output segments — all in f32 on the host."""
    w_c = attn_w[:D].astype(np.float32)
    out = np.empty((B, L, 4 * D), np.float32)
    out[:, :, 0:D] = context
    for i in range(NCORES):
        sl = slice(i * BPC, (i + 1) * BPC)
        ctx_i = context[sl]
        r = results[i]["out3"].astype(np.float32)               # [BPC,128,NOUT]
        tiles = r[:, :, :O_EB].reshape(BPC, 128, NT, TW)
        a = tiles[..., :D] / tiles[..., D : D + 1]
        a = a.transpose(0, 2, 1, 3).reshape(BPC, L, D)          # un-permute l
        # beta = softmax_l(q2c);  q2c = log(eb) + s_c  (attn_b, s_q-max
        # terms constant per batch cancel; s_q rode the device exp)
        q2c = np.log(r[:, :, O_EB:]).transpose(0, 2, 1).reshape(BPC, L)
        q2c += ctx_i @ w_c
        q2c -= q2c.max(axis=1, keepdims=True)
        ebf = np.exp(q2c)
        beta = ebf / ebf.sum(axis=1, keepdims=True)
        bvec = np.einsum('bl,bld->bd', beta, ctx_i)
        out[sl, :, D : 2 * D] = a
        out[sl, :, 2 * D : 3 * D] = ctx_i * a
        out[sl, :, 3 * D : 4 * D] = ctx_i * bvec[:, None, :]
    return out


_NC_CACHE: list = []


def kernel(**inputs: np.ndarray) -> np.ndarray:
    context = np.ascontiguousarray(np.asarray(inputs["context"], np.float32))
    query = np.ascontiguousarray(np.asarray(inputs["query"], np.float32))
    attn_w = np.ascontiguousarray(np.asarray(inputs["attn_w"], np.float32))

    if not _NC_CACHE:
        _NC_CACHE.append(build_module())
    nc = _NC_CACHE[0]

    core_ids = list(range(NCORES))
    res = run_bass_kernel_spmd(nc, make_in_maps(context, query, attn_w), core_ids)
    return assemble(context, attn_w, res.results)


if __name__ == "__main__":
    rng = np.random.default_rng(0)
    inputs = {
        "context": rng.standard_normal((B, L, D), dtype=np.float32),
        "context_masks": np.ones((B, L), np.float32),
        "query": rng.standard_normal((B, Q, D), dtype=np.float32),
        "query_masks": np.ones((B, Q), np.float32),
        "attn_w": (rng.standard_normal(3 * D) * 0.05).astype(np.float32),
        "attn_b": (rng.standard_normal(1) * 0.05).astype(np.float32),
    }
    out = kernel(**inputs)
    print("out", out.shape, out.dtype)


# revision 12
# speedup vs baseline: 1.1126x; 1.1126x over previous
"""Trainium2 Bass kernel for the BiDAF-style trilinear attention module.

Math (per batch b; bf16 operands, f32 PSUM accumulate — harness gate is
rel_err < 2e-2):
  w_c, w_q, w_cq = attn_w[0:256], attn_w[256:512], attn_w[512:768]
  sim[l,q] = ctx[l]·w_c + qry[q]·w_q + (ctx[l]*w_cq)·qry[q] + attn_b
  alpha    = softmax_q(sim)                      (masks are all-ones)
  a        = alpha @ qry                         [L, D]
  q2c      = max_q(sim);  beta = softmax_l(q2c)
  bvec     = beta @ ctx                          [D]
  out      = concat([ctx, a, ctx*a, ctx*bvec])   [L, 4D]

The device computes ONLY the parts that need the big L×Q similarity
matrix; everything cheap rides the host gather step, so HBM traffic
per core drops to 9.1 MB and per-engine work per batch stays under the
DMA budget:
  * attn_b cancels in both softmaxes -> dropped entirely.
  * s_c[l] = ctx[l]·w_c is constant along the softmax_q axis -> alpha
    does not need it; the host adds it back for the beta softmax.
  * qext = qryT * w_cq (the sim stationary operand) and s_q = qry·w_q
    are precomputed on the HOST: qext ships in place of qryT, s_q
    ships as a tiny f32 tensor that becomes the ACT exp bias.
  * sim' is computed TRANSPOSED (simT[q,l]) with a 512-wide moving dim:
    2 accumulating matmuls per 512-block; alphaU = exp(simT' + s_q)
    straight from PSUM (ACT, per-partition bias).
  * alpha rowsum rides as a 257th ones-column of the a-matmul rhs; the
    device ships UNNORMALIZED a' and the rowsum s; the host divides.
  * the a-matmuls write PAIRS of l-tiles into one 2-bank PSUM tile
    (bank-aligned 257-col slices) so each PSUM->SBUF copy covers two
    tiles — half the per-op fixed overhead on ACT/DVE.
  * eb[l] = max_q alphaU = exp(q2c[l] - s_c[l] - s_q-less terms) -> one
    PE transpose per 128-l tile (4 per PSUM bank) + one free-axis DVE
    max per block; host: beta = softmax_l(log(eb) + s_c), bvec = beta@ctx.
  * out[:, :, 0:D] == ctx (exact), ctx*a and ctx*bvec are elementwise
    f32 products done on the host during the gather.

I/O layout: the host packs per-batch inputs (ctx transposed, qext,
qry+ones) into ONE partition-major bf16 buffer so each batch is a
single fully-contiguous 0.63 MB DMA (5.1 KB/partition lines); the
device writes one 2064-col bf16 output buffer per batch ([a'|s] tiles
+ eb), shipped as two ~260 KB DMAs (2 KB lines) — per-pair DMAs on the
last batch to shorten the drain tail.

Sharding: data-parallel over batch, 8 batches per NeuronCore x 8 cores.
"""

import sys

sys.path.insert(0, "/opt/trn_rl_repo")

from contextlib import ExitStack

import numpy as np
import ml_dtypes

import concourse.bass as bass
import concourse.bacc as bacc
import concourse.tile as tile
from concourse import mybir
from concourse.masks import make_identity
from concourse.bass_utils import run_bass_kernel_spmd

B, L, Q, D = 64, 1024, 128, 256
NCORES = 8
BPC = B // NCORES          # batches per core
NT = L // 128              # 128-row l-tiles per batch
BW = 512                   # sim block width (l columns per PSUM bank)
NBLK = L // BW             # sim blocks per batch
TPB = BW // 128            # l-tiles per sim block
F32 = mybir.dt.float32
BF16 = mybir.dt.bfloat16
EXP = mybir.ActivationFunctionType.Exp
X = mybir.AxisListType.X
NPBF16 = ml_dtypes.bfloat16

# packed input layout (per batch, per partition, bf16 elements):
#   [ ctxT (c,l) | qext = (qry*w_cq)T (c,q) | qry row + ones ]
O_CT = 0                   # ctxT [128, 2, L]
O_QE = O_CT + 2 * L        # qextT [128, 2, Q]
O_QN = O_QE + 2 * Q        # [qry | 1] [128, D+1]
NIN = O_QN + D + 1         # 2561

# output layout (per batch, per partition, bf16 elements):
#   [ NT tiles of [a'(256) | rowsum(1)] | eb (NT) ]
TW = D + 1                 # a-tile width incl rowsum rider
O_EB = NT * TW             # 2056
NOUT = O_EB + NT           # 2064
BLKW = TPB * TW            # output cols per sim block (1028)


def build_module() -> bass.Bass:
    # Bacc (not plain Bass): its compile() pass splits multi-sem waits into
    # event semaphores — walrus's LDWEIGHTS struct only carries one wait.
    nc = bacc.Bacc("TRN2", target_bir_lowering=False)
    in_t = nc.declare_dram_parameter("inpack", [BPC, 128, NIN], BF16, isOutput=False)
    sq_t = nc.declare_dram_parameter("sq_all", [128, BPC], F32, isOutput=False)
    id_t = nc.declare_dram_parameter("ident", [128, 128], BF16, isOutput=False)
    out_t = nc.declare_dram_parameter("out3", [BPC, 128, NOUT], BF16,
                                      isOutput=True)

    with tile.TileContext(nc) as tc, ExitStack() as ctx:
        consts = ctx.enter_context(tc.tile_pool(name="consts", bufs=1))
        sb = ctx.enter_context(tc.tile_pool(name="sb", bufs=6))
        # dedicated SBUF buffers for every batch — zero pool-reuse waits
        big = ctx.enter_context(tc.tile_pool(name="big", bufs=BPC))
        ob = ctx.enter_context(tc.tile_pool(name="ob", bufs=BPC))
        # PSUM: 8 banks exactly — sim(2x1) + at(2x1) + a-pairs(2x2)
        ps_sim = ctx.enter_context(tc.tile_pool(name="ps_sim", bufs=2, space="PSUM"))
        ps_at = ctx.enter_context(tc.tile_pool(name="ps_at", bufs=2, space="PSUM"))
        ps_a = ctx.enter_context(tc.tile_pool(name="ps_a", bufs=2, space="PSUM"))

        # identity (for PE transposes) arrives by DMA so the GpSimd queue
        # stays free for output-DMA issue from t=0
        identity = consts.tile([128, 128], BF16)
        nc.sync.dma_start(out=identity, in_=id_t[:, :])
        # s_q for all batches as ACT-bias columns, one tiny DMA
        sqsb = consts.tile([128, BPC], F32)
        nc.sync.dma_start(out=sqsb, in_=sq_t[:, :])

        # PE warm-up: dummy matmuls while the first input DMAs are in
        # flight, so the HAM clock ramp completes before the real work.
        wtile = ps_at.tile([128, 128], F32, tag="at", name="warmup")
        for _ in range(24):
            nc.tensor.matmul(wtile, lhsT=identity, rhs=identity,
                             start=True, stop=True)

        def dma_in(b):
            # dual-ring traffic split: ALL inputs ride the Sync HWDGE
            # ring (issued upfront, zero waits), all outputs ride the
            # GpSimd software-DGE ring, so in/out transfers overlap.
            ibuf = big.tile([128, NIN], BF16, tag="ibuf", name=f"ibuf{b}")
            if b == 0:
                # split so the first sim block starts before qn lands
                nc.sync.dma_start(out=ibuf[:, O_CT:O_QN], in_=in_t[b][:, O_CT:O_QN])
                nc.sync.dma_start(out=ibuf[:, O_QN:NIN], in_=in_t[b][:, O_QN:NIN])
            else:
                nc.sync.dma_start(out=ibuf, in_=in_t[b])
            return {
                "ct2": ibuf[:, O_CT:O_QE].rearrange("p (c l) -> p c l", c=2),
                "qe2": ibuf[:, O_QE:O_QN].rearrange("p (c q) -> p c q", c=2),
                "qn": ibuf[:, O_QN:NIN],
            }

        def sim_block(b, st, j):
            ct2, qe2 = st["ct2"], st["qe2"]
            lo, hi = j * BW, (j + 1) * BW
            # simT'[q, l] = (qry*w_cq)·ctx — no s_c fold (host adds it)
            sim_ps = ps_sim.tile([128, BW], F32, tag="sim", name=f"sim{b}_{j}")
            nc.tensor.matmul(sim_ps, lhsT=qe2[:, 0, :], rhs=ct2[:, 0, lo:hi],
                             start=True, stop=False)
            nc.tensor.matmul(sim_ps, lhsT=qe2[:, 1, :], rhs=ct2[:, 1, lo:hi],
                             start=False, stop=True)
            # alphaU[q, l] = exp(simT' + s_q) — unnormalized alpha^T
            alphaU = sb.tile([128, BW], BF16, tag="alpha", name=f"alpha{b}_{j}")
            nc.scalar.activation(out=alphaU, in_=sim_ps, func=EXP,
                                 bias=sqsb[:, b : b + 1])
            st[f"alpha{j}"] = alphaU

        def tail_block(b, st, j):
            qn, obuf, alphaU = st["qn"], st["obuf"], st[f"alpha{j}"]
            at_ps = st["at_ps"]
            t0 = j * TPB
            # eb[l] = max over q of alphaU — PE transposes of the whole
            # batch pack one PSUM bank, then a SINGLE per-batch DVE max
            for i in range(TPB):
                nc.tensor.transpose(at_ps[:, t0 + i, :],
                                    alphaU[:, i * 128 : (i + 1) * 128], identity)
            if j == NBLK - 1:
                nc.vector.reduce_max(obuf[:, O_EB : O_EB + NT], at_ps, axis=X)
            for p in range(TPB // 2):
                # a-matmul PAIR: two l-tiles into one 2-bank PSUM tile,
                # each [a' | rowsum] slice bank-aligned (257 of 512 cols)
                a_ps = ps_a.tile([128, 2, 512], F32, tag="a",
                                 name=f"a_ps{b}_{j}_{p}")
                for i in range(2):
                    asl = alphaU[:, (2 * p + i) * 128 : (2 * p + i + 1) * 128]
                    nc.tensor.matmul(a_ps[:, i, 0:TW], lhsT=asl, rhs=qn,
                                     start=True, stop=True)
                # one 514-col PSUM->SBUF copy per pair, ACT/DVE alternating
                dst = obuf[:, (t0 + 2 * p) * TW : (t0 + 2 * p + 2) * TW]
                dst = dst.rearrange("p (i w) -> p i w", i=2)
                if p == 0:
                    nc.scalar.copy(dst, a_ps[:, :, 0:TW])
                else:
                    nc.vector.tensor_copy(dst, a_ps[:, :, 0:TW])
                if b == BPC - 1:
                    # drain tail: ship each pair as soon as it is copied
                    lo = (t0 + 2 * p) * TW
                    hi = (t0 + 2 * p + 2) * TW if (j, p) != (NBLK - 1, 1) \
                        else NOUT
                    nc.gpsimd.dma_start(out=out_t[b][:, lo:hi],
                                        in_=obuf[:, lo:hi])
            if b < BPC - 1:
                # per-block output DMA (block 1 carries the eb tail columns)
                lo = j * BLKW
                hi = (j + 1) * BLKW if j < NBLK - 1 else NOUT
                nc.gpsimd.dma_start(out=out_t[b][:, lo:hi], in_=obuf[:, lo:hi])

        # Software pipeline: input DMAs prefetched two batches ahead; both
        # sim blocks are emitted before either tail so the PE never waits
        # on the ACT exp.
        # all inputs issued up front (dedicated buffers -> no waits)
        states = {b: dma_in(b) for b in range(BPC)}
        for b in range(BPC):
            st = states.pop(b)
            st["obuf"] = ob.tile([128, NOUT], BF16, tag="obuf",
                                 name=f"obuf{b}")
            st["at_ps"] = ps_at.tile([128, NT, 128], BF16, tag="at",
                                     name=f"at{b}")
            sim_block(b, st, 0)
            sim_block(b, st, 1)
            tail_block(b, st, 0)
            tail_block(b, st, 1)

    nc.finalize()
    return nc


def make_in_maps(context: np.ndarray, query: np.ndarray, attn_w: np.ndarray):
    """Shard + lay out the full f32 inputs for the 8 cores: one packed
    partition-major bf16 buffer per batch (see layout comment up top),
    plus the per-batch s_q bias columns in f32."""
    w_cq = attn_w[2 * D :].astype(np.float32)
    w_q = attn_w[D : 2 * D].astype(np.float32)
    ctx_b = context.astype(NPBF16)
    qe_b = (query * w_cq).astype(NPBF16)       # qext, host-side
    qry_b = query.astype(NPBF16)
    sq = (query.astype(np.float32) @ w_q).astype(np.float32)   # [B, Q]
    maps = []
    for i in range(NCORES):
        sl = slice(i * BPC, (i + 1) * BPC)
        c = ctx_b[sl]                                          # [BPC, L, D]
        qe = qe_b[sl]                                          # [BPC, Q, D]
        c2 = np.ascontiguousarray(c.transpose(0, 2, 1)).reshape(
            BPC, 2, 128, L).transpose(0, 2, 1, 3).reshape(BPC, 128, 2 * L)
        qT = np.ascontiguousarray(qe.transpose(0, 2, 1)).reshape(
            BPC, 2, 128, Q).transpose(0, 2, 1, 3).reshape(BPC, 128, 2 * Q)
        qn = np.concatenate([qry_b[sl], np.ones((BPC, Q, 1), NPBF16)], axis=2)
        inpack = np.ascontiguousarray(
            np.concatenate([c2, qT, qn], axis=2))              # [BPC, 128, NIN]
        maps.append({"inpack": inpack,
                     "sq_all": np.ascontiguousarray(sq[sl].T),   # [128, BPC]
                     "ident": np.eye(128, dtype=NPBF16)})
    return maps


def assemble(context: np.ndarray, attn_w: np.ndarray, results) -> np.ndarray:
    """Gather per-core [a'|s] tiles + eb, normalize a, rebuild the beta
    path (softmax_l(log eb + s_c), bvec = beta@ctx) and the elementwise
    output segments — all in f32 on the host."""
    w_c = attn_w[:D].astype(np.float32)
    out = np.empty((B, L, 4 * D), np.float32)
    out[:, :, 0:D] = context
    for i in range(NCORES):
        sl = slice(i * BPC, (i + 1) * BPC)
        ctx_i = context[sl]
        r = results[i]["out3"].astype(np.float32)               # [BPC,128,NOUT]
        tiles = r[:, :, :O_EB].reshape(BPC, 128, NT, TW)
        a = tiles[..., :D] / tiles[..., D : D + 1]
        a = a.transpose(0, 2, 1, 3).reshape(BPC, L, D)          # un-permute l
        # beta = softmax_l(q2c);  q2c = log(eb) + s_c  (attn_b, s_q-max
        # terms constant per batch cancel; s_q rode the device exp)
        q2c = np.log(r[:, :, O_EB:]).transpose(0, 2, 1).reshape(BPC, L)
        q2c += ctx_i @ w_c
        q2c -= q2c.max(axis=1, keepdims=True)
        ebf = np.exp(q2c)
        beta = ebf / ebf.sum(axis=1, keepdims=True)
        bvec = np.einsum('bl,bld->bd', beta, ctx_i)
        out[sl, :, D : 2 * D] = a
        out[sl, :, 2 * D : 3 * D] = ctx_i * a
        out[sl, :, 3 * D : 4 * D] = ctx_i * bvec[:, None, :]
    return out


_NC_CACHE: list = []


def kernel(**inputs: np.ndarray) -> np.ndarray:
    context = np.ascontiguousarray(np.asarray(inputs["context"], np.float32))
    query = np.ascontiguousarray(np.asarray(inputs["query"], np.float32))
    attn_w = np.ascontiguousarray(np.asarray(inputs["attn_w"], np.float32))

    if not _NC_CACHE:
        _NC_CACHE.append(build_module())
    nc = _NC_CACHE[0]

    core_ids = list(range(NCORES))
    res = run_bass_kernel_spmd(nc, make_in_maps(context, query, attn_w), core_ids)
    return assemble(context, attn_w, res.results)


if __name__ == "__main__":
    rng = np.random.default_rng(0)
    inputs = {
        "context": rng.standard_normal((B, L, D), dtype=np.float32),
        "context_masks": np.ones((B, L), np.float32),
        "query": rng.standard_normal((B, Q, D), dtype=np.float32),
        "query_masks": np.ones((B, Q), np.float32),
        "attn_w": (rng.standard_normal(3 * D) * 0.05).astype(np.float32),
        "attn_b": (rng.standard_normal(1) * 0.05).astype(np.float32),
    }
    out = kernel(**inputs)
    print("out", out.shape, out.dtype)


# revision 14
# speedup vs baseline: 1.1579x; 1.0407x over previous
"""Trainium2 Bass kernel for the BiDAF-style trilinear attention module.

Math (per batch b; bf16 operands, f32 PSUM accumulate — harness gate is
rel_err < 2e-2):
  w_c, w_q, w_cq = attn_w[0:256], attn_w[256:512], attn_w[512:768]
  sim[l,q] = ctx[l]·w_c + qry[q]·w_q + (ctx[l]*w_cq)·qry[q] + attn_b
  alpha    = softmax_q(sim)                      (masks are all-ones)
  a        = alpha @ qry                         [L, D]
  q2c      = max_q(sim);  beta = softmax_l(q2c)
  bvec     = beta @ ctx                          [D]
  out      = concat([ctx, a, ctx*a, ctx*bvec])   [L, 4D]

The device computes ONLY the parts that need the big L×Q similarity
matrix; everything cheap rides the host gather step, so HBM traffic
per core drops to 9.1 MB and per-engine work per batch stays under the
DMA budget:
  * attn_b cancels in both softmaxes -> dropped entirely.
  * s_c[l] = ctx[l]·w_c is constant along the softmax_q axis -> alpha
    does not need it; the host adds it back for the beta softmax.
  * qext = qryT * w_cq (the sim stationary operand) and s_q = qry·w_q
    are precomputed on the HOST: qext ships in place of qryT, s_q
    ships as a tiny f32 tensor that becomes the ACT exp bias.
  * sim' is computed TRANSPOSED (simT[q,l]) with a 512-wide moving dim:
    2 accumulating matmuls per 512-block; alphaU = exp(simT' + s_q)
    straight from PSUM (ACT, per-partition bias).
  * alpha rowsum rides as a 257th ones-column of the a-matmul rhs; the
    device ships UNNORMALIZED a' and the rowsum s; the host divides.
  * the a-matmuls write PAIRS of l-tiles into one 2-bank PSUM tile
    (bank-aligned 257-col slices) so each PSUM->SBUF copy covers two
    tiles — half the per-op fixed overhead on ACT/DVE.
  * eb[l] = max_q alphaU = exp(q2c[l] - s_c[l] - s_q-less terms) -> one
    PE transpose per 128-l tile (4 per PSUM bank) + one free-axis DVE
    max per block; host: beta = softmax_l(log(eb) + s_c), bvec = beta@ctx.
  * out[:, :, 0:D] == ctx (exact), ctx*a and ctx*bvec are elementwise
    f32 products done on the host during the gather.

I/O layout: the host packs per-batch inputs (ctx transposed, qext,
qry+ones) into ONE partition-major bf16 buffer so each batch is a
single fully-contiguous 0.63 MB DMA (5.1 KB/partition lines); the
device writes one 2064-col bf16 output buffer per batch ([a'|s] tiles
+ eb), shipped as two ~260 KB DMAs (2 KB lines) — per-pair DMAs on the
last batch to shorten the drain tail.

Sharding: data-parallel over batch, 8 batches per NeuronCore x 8 cores.
"""

import sys

sys.path.insert(0, "/opt/trn_rl_repo")

from contextlib import ExitStack

import numpy as np
import ml_dtypes

import concourse.bass as bass
import concourse.bacc as bacc
import concourse.tile as tile
from concourse import mybir
from concourse.masks import make_identity
from concourse.bass_utils import run_bass_kernel_spmd

B, L, Q, D = 64, 1024, 128, 256
NCORES = 8
BPC = B // NCORES          # batches per core
NT = L // 128              # 128-row l-tiles per batch
BW = 512                   # sim block width (l columns per PSUM bank)
NBLK = L // BW             # sim blocks per batch
TPB = BW // 128            # l-tiles per sim block
F32 = mybir.dt.float32
BF16 = mybir.dt.bfloat16
EXP = mybir.ActivationFunctionType.Exp
X = mybir.AxisListType.X
NPBF16 = ml_dtypes.bfloat16

# packed input layout (per batch, per partition, bf16 elements):
#   [ ctxT (c,l) | qext = (qry*w_cq)T (c,q) | qry row + ones ]
O_CT = 0                   # ctxT [128, 2, L]
O_QE = O_CT + 2 * L        # qextT [128, 2, Q]
O_QN = O_QE + 2 * Q        # [qry | 1] [128, D+1]
NIN = O_QN + D + 1         # 2561

# output layout (per batch, per partition, bf16 elements):
#   [ NT tiles of [a'(256) | rowsum(1)] | eb (NT) ]
TW = D + 1                 # a-tile width incl rowsum rider
O_EB = NT * TW             # 2056
NOUT = O_EB + NT           # 2064
BLKW = TPB * TW            # output cols per sim block (1028)


def build_module() -> bass.Bass:
    # Bacc (not plain Bass): its compile() pass splits multi-sem waits into
    # event semaphores — walrus's LDWEIGHTS struct only carries one wait.
    nc = bacc.Bacc("TRN2", target_bir_lowering=False)
    in_t = nc.declare_dram_parameter("inpack", [BPC, 128, NIN], BF16, isOutput=False)
    sq_t = nc.declare_dram_parameter("sq_all", [128, BPC], F32, isOutput=False)
    id_t = nc.declare_dram_parameter("ident", [128, 128], BF16, isOutput=False)
    out_t = nc.declare_dram_parameter("out3", [BPC, 128, NOUT], BF16,
                                      isOutput=True)

    with tile.TileContext(nc) as tc, ExitStack() as ctx:
        consts = ctx.enter_context(tc.tile_pool(name="consts", bufs=1))
        sb = ctx.enter_context(tc.tile_pool(name="sb", bufs=6))
        # dedicated SBUF buffers for every batch — zero pool-reuse waits
        big = ctx.enter_context(tc.tile_pool(name="big", bufs=BPC))
        ob = ctx.enter_context(tc.tile_pool(name="ob", bufs=BPC))
        # PSUM: 8 banks exactly — sim(2x1) + at(2x1) + a-pairs(2x2)
        ps_sim = ctx.enter_context(tc.tile_pool(name="ps_sim", bufs=2, space="PSUM"))
        ps_at = ctx.enter_context(tc.tile_pool(name="ps_at", bufs=2, space="PSUM"))
        ps_a = ctx.enter_context(tc.tile_pool(name="ps_a", bufs=2, space="PSUM"))

        # identity (for PE transposes) arrives by DMA so the GpSimd queue
        # stays free for output-DMA issue from t=0
        identity = consts.tile([128, 128], BF16)
        nc.sync.dma_start(out=identity, in_=id_t[:, :])

        def dma_in(b):
            # dual-ring traffic split: ALL inputs ride the Sync HWDGE
            # ring (issued upfront, zero waits), all outputs ride the
            # GpSimd software-DGE ring, so in/out transfers overlap.
            ibuf = big.tile([128, NIN], BF16, tag="ibuf", name=f"ibuf{b}")
            nc.sync.dma_start(out=ibuf, in_=in_t[b])
            return {
                "ct2": ibuf[:, O_CT:O_QE].rearrange("p (c l) -> p c l", c=2),
                "qe2": ibuf[:, O_QE:O_QN].rearrange("p (c q) -> p c q", c=2),
                "qn": ibuf[:, O_QN:NIN],
            }

        # batch 0's input goes out right behind the identity tile so the
        # first sim block starts as early as possible
        states = {0: dma_in(0)}
        # s_q for all batches as ACT-bias columns, one tiny DMA
        sqsb = consts.tile([128, BPC], F32)
        nc.sync.dma_start(out=sqsb, in_=sq_t[:, :])

        # PE warm-up: dummy matmuls while the first input DMAs are in
        # flight, to trigger the HAM clock ramp before the real work.
        wtile = ps_at.tile([128, 128], F32, tag="at", name="warmup")
        for _ in range(10):
            nc.tensor.matmul(wtile, lhsT=identity, rhs=identity,
                             start=True, stop=True)

        def sim_block(b, st, j):
            ct2, qe2 = st["ct2"], st["qe2"]
            lo, hi = j * BW, (j + 1) * BW
            # simT'[q, l] = (qry*w_cq)·ctx — no s_c fold (host adds it)
            sim_ps = ps_sim.tile([128, BW], F32, tag="sim", name=f"sim{b}_{j}")
            nc.tensor.matmul(sim_ps, lhsT=qe2[:, 0, :], rhs=ct2[:, 0, lo:hi],
                             start=True, stop=False)
            nc.tensor.matmul(sim_ps, lhsT=qe2[:, 1, :], rhs=ct2[:, 1, lo:hi],
                             start=False, stop=True)
            # alphaU[q, l] = exp(simT' + s_q) — unnormalized alpha^T
            alphaU = sb.tile([128, BW], BF16, tag="alpha", name=f"alpha{b}_{j}")
            nc.scalar.activation(out=alphaU, in_=sim_ps, func=EXP,
                                 bias=sqsb[:, b : b + 1])
            st[f"alpha{j}"] = alphaU

        def tail_block(b, st, j):
            qn, obuf, alphaU = st["qn"], st["obuf"], st[f"alpha{j}"]
            at_ps = st["at_ps"]
            t0 = j * TPB
            # eb[l] = max over q of alphaU — PE transposes of the whole
            # batch pack one PSUM bank, then a SINGLE per-batch DVE max
            for i in range(TPB):
                nc.tensor.transpose(at_ps[:, t0 + i, :],
                                    alphaU[:, i * 128 : (i + 1) * 128], identity)
            if j == NBLK - 1:
                nc.vector.reduce_max(obuf[:, O_EB : O_EB + NT], at_ps, axis=X)
            for p in range(TPB // 2):
                # a-matmul PAIR: two l-tiles into one 2-bank PSUM tile,
                # each [a' | rowsum] slice bank-aligned (257 of 512 cols)
                a_ps = ps_a.tile([128, 2, 512], F32, tag="a",
                                 name=f"a_ps{b}_{j}_{p}")
                for i in range(2):
                    asl = alphaU[:, (2 * p + i) * 128 : (2 * p + i + 1) * 128]
                    nc.tensor.matmul(a_ps[:, i, 0:TW], lhsT=asl, rhs=qn,
                                     start=True, stop=True)
                # one 514-col PSUM->SBUF copy per pair, ACT/DVE alternating
                dst = obuf[:, (t0 + 2 * p) * TW : (t0 + 2 * p + 2) * TW]
                dst = dst.rearrange("p (i w) -> p i w", i=2)
                if p == 0:
                    nc.scalar.copy(dst, a_ps[:, :, 0:TW])
                else:
                    nc.vector.tensor_copy(dst, a_ps[:, :, 0:TW])
                if b == BPC - 1:
                    # drain tail: ship each pair as soon as it is copied
                    lo = (t0 + 2 * p) * TW
                    hi = (t0 + 2 * p + 2) * TW if (j, p) != (NBLK - 1, 1) \
                        else NOUT
                    nc.gpsimd.dma_start(out=out_t[b][:, lo:hi],
                                        in_=obuf[:, lo:hi])
            if b < BPC - 1:
                # per-block output DMA (block 1 carries the eb tail columns)
                lo = j * BLKW
                hi = (j + 1) * BLKW if j < NBLK - 1 else NOUT
                nc.gpsimd.dma_start(out=out_t[b][:, lo:hi], in_=obuf[:, lo:hi])

        # Software pipeline: input DMAs prefetched two batches ahead; both
        # sim blocks are emitted before either tail so the PE never waits
        # on the ACT exp.
        # remaining inputs issued up front (dedicated buffers -> no waits)
        for b in range(1, BPC):
            states[b] = dma_in(b)
        for b in range(BPC):
            st = states.pop(b)
            st["obuf"] = ob.tile([128, NOUT], BF16, tag="obuf",
                                 name=f"obuf{b}")
            st["at_ps"] = ps_at.tile([128, NT, 128], BF16, tag="at",
                                     name=f"at{b}")
            sim_block(b, st, 0)
            sim_block(b, st, 1)
            tail_block(b, st, 0)
            tail_block(b, st, 1)

    nc.finalize()
    return nc


def make_in_maps(context: np.ndarray, query: np.ndarray, attn_w: np.ndarray):
    """Shard + lay out the full f32 inputs for the 8 cores: one packed
    partition-major bf16 buffer per batch (see layout comment up top),
    plus the per-batch s_q bias columns in f32."""
    w_cq = attn_w[2 * D :].astype(np.float32)
    w_q = attn_w[D : 2 * D].astype(np.float32)
    ctx_b = context.astype(NPBF16)
    qe_b = (query * w_cq).astype(NPBF16)       # qext, host-side
    qry_b = query.astype(NPBF16)
    sq = (query.astype(np.float32) @ w_q).astype(np.float32)   # [B, Q]
    maps = []
    for i in range(NCORES):
        sl = slice(i * BPC, (i + 1) * BPC)
        c = ctx_b[sl]                                          # [BPC, L, D]
        qe = qe_b[sl]                                          # [BPC, Q, D]
        c2 = np.ascontiguousarray(c.transpose(0, 2, 1)).reshape(
            BPC, 2, 128, L).transpose(0, 2, 1, 3).reshape(BPC, 128, 2 * L)
        qT = np.ascontiguousarray(qe.transpose(0, 2, 1)).reshape(
            BPC, 2, 128, Q).transpose(0, 2, 1, 3).reshape(BPC, 128, 2 * Q)
        qn = np.concatenate([qry_b[sl], np.ones((BPC, Q, 1), NPBF16)], axis=2)
        inpack = np.ascontiguousarray(
            np.concatenate([c2, qT, qn], axis=2))              # [BPC, 128, NIN]
        maps.append({"inpack": inpack,
                     "sq_all": np.ascontiguousarray(sq[sl].T),   # [128, BPC]
                     "ident": np.eye(128, dtype=NPBF16)})
    return maps


def assemble(context: np.ndarray, attn_w: np.ndarray, results) -> np.ndarray:
    """Gather per-core [a'|s] tiles + eb, normalize a, rebuild the beta
    path (softmax_l(log eb + s_c), bvec = beta@ctx) and the elementwise
    output segments — all in f32 on the host."""
    w_c = attn_w[:D].astype(np.float32)
    out = np.empty((B, L, 4 * D), np.float32)
    out[:, :, 0:D] = context
    for i in range(NCORES):
        sl = slice(i * BPC, (i + 1) * BPC)
        ctx_i = context[sl]
        r = results[i]["out3"].astype(np.float32)               # [BPC,128,NOUT]
        tiles = r[:, :, :O_EB].reshape(BPC, 128, NT, TW)
        a = tiles[..., :D] / tiles[..., D : D + 1]
        a = a.transpose(0, 2, 1, 3).reshape(BPC, L, D)          # un-permute l
        # beta = softmax_l(q2c);  q2c = log(eb) + s_c  (attn_b, s_q-max
        # terms constant per batch cancel; s_q rode the device exp)
        q2c = np.log(r[:, :, O_EB:]).transpose(0, 2, 1).reshape(BPC, L)
        q2c += ctx_i @ w_c
        q2c -= q2c.max(axis=1, keepdims=True)
        ebf = np.exp(q2c)
        beta = ebf / ebf.sum(axis=1, keepdims=True)
        bvec = np.einsum('bl,bld->bd', beta, ctx_i)
        out[sl, :, D : 2 * D] = a
        out[sl, :, 2 * D : 3 * D] = ctx_i * a
        out[sl, :, 3 * D : 4 * D] = ctx_i * bvec[:, None, :]
    return out


_NC_CACHE: list = []


def kernel(**inputs: np.ndarray) -> np.ndarray:
    context = np.ascontiguousarray(np.asarray(inputs["context"], np.float32))
    query = np.ascontiguousarray(np.asarray(inputs["query"], np.float32))
    attn_w = np.ascontiguousarray(np.asarray(inputs["attn_w"], np.float32))

    if not _NC_CACHE:
        _NC_CACHE.append(build_module())
    nc = _NC_CACHE[0]

    core_ids = list(range(NCORES))
    res = run_bass_kernel_spmd(nc, make_in_maps(context, query, attn_w), core_ids)
    return assemble(context, attn_w, res.results)


if __name__ == "__main__":
    rng = np.random.default_rng(0)
    inputs = {
        "context": rng.standard_normal((B, L, D), dtype=np.float32),
        "context_masks": np.ones((B, L), np.float32),
        "query": rng.standard_normal((B, Q, D), dtype=np.float32),
        "query_masks": np.ones((B, Q), np.float32),
        "attn_w": (rng.standard_normal(3 * D) * 0.05).astype(np.float32),
        "attn_b": (rng.standard_normal(1) * 0.05).astype(np.float32),
    }
    out = kernel(**inputs)
    print("out", out.shape, out.dtype)
